# revision 12
# baseline (speedup 1.0000x reference)
"""GCN 2-layer kernel for TRN2 x8 cores — host prep + Bass/Tile builder.

Math: out1 = relu(dinv ⊙ (Aᵀ (dinv ⊙ x)) @ W1 + b1)
      out2 = relu(dinv ⊙ (Aᵀ (dinv ⊙ out1)) @ W2 + b2)
with A = adjacency incl. self-loops, dinv = rsqrt(in-degree incl self).
(W1/W2 commute with the segment-sum, so both are applied AFTER the
per-window aggregation — layer 1 gathers raw x~ rows, not x~@W1.)

Device plan (SPMD, 8 cores, one program), v9:
- nodes dst-sharded by core (NPC per core, BLK = padded block).
- src nodes split into two ranges BY LOCAL ROW, not by core:
  range A = local rows [0, ASPL), range B = [ASPL, BLK). Table row =
  c*ASPL + r (A) or c*BSPL + (r - ASPL) (B). Both ranges stay int16-
  addressable, and each range's layer-2 table is a SEPARATE Shared
  tensor filled by its own AllGather: AG-A fires as soon as layer-1
  windows covering rows < ASPL are done (hidden under the remaining
  windows); AG-B fires at layer-1 end (only ~35us exposed).
- layer-1 gather source = xrows (replicated input, the same two-range
  row layout). No device-side table build — gathers start at ~10us.
- edges bucketed by (src range, dst 128-col sub-window); chunks of
  128 edges; chunk counts shared across cores (max over cores); pad
  slots gather row 0 with a zero one-hot row.
- messages fetched with dma_gather (bf16 256B rows) round-robin over
  4 SWDGE queues (descriptor generation on Q7 core pairs is the
  bottleneck). msgs pool 4 deep per range so the message-tile WAR
  doesn't gate gather issue.
- one-hot S streamed from DRAM; segment-sum via PE (msgs stationary,
  S moving, PSUM window accumulation); self-loop opens each window.
- layer1 tail: zxb=bf16(zx); z1=W1ᵀ@zxb; h2b=dinv*relu(dinv*z1+b1);
  transpose -> ag_in slab. layer2 tail: out = relu(W2ᵀ(dinv*zh)+b2).
"""
import numpy as np
import ml_dtypes

BF16 = ml_dtypes.bfloat16


# ---------------------------------------------------------------- structure
class Struct:
    pass


def make_structure(N, NC, WIN=512, SUB=128):
    P = Struct()
    P.N, P.NC, P.WIN, P.SUB = N, NC, WIN, SUB
    assert N % NC == 0
    P.NPC = N // NC
    P.BLK = ((P.NPC + 1 + 31) // 32) * 32
    assert P.BLK % SUB == 0 and P.BLK % 128 == 0
    P.windows = []
    col0 = 0
    while col0 < P.BLK:
        ncols = min(WIN, P.BLK - col0)
        w = Struct()
        w.col0, w.ncols = col0, ncols
        w.sw0, w.nsw = col0 // SUB, ncols // SUB
        P.windows.append(w)
        col0 += ncols
    # split source rows at a window boundary near BLK/2
    P.AG_SPLIT_WIN = len(P.windows) // 2  # windows [0, this) are range A
    P.ASPL = P.windows[P.AG_SPLIT_WIN].col0  # local rows in range A
    P.BSPL = P.BLK - P.ASPL
    P.NA = NC * P.ASPL  # total range-A table rows
    P.NB = NC * P.BSPL
    assert P.NA <= 32768 and P.NB <= 32768
    P.NSW = P.BLK // SUB
    return P


# ---------------------------------------------------------------- host prep
def prep(P, x, edge_index, W1, b1, W2, b2):
    """Returns in_maps — the per-core input dict list. Also fills P.plan."""
    N, NC, NPC, BLK, SUB = P.N, P.NC, P.NPC, P.BLK, P.SUB
    F = x.shape[1]
    HID = W1.shape[1]
    DOUT = W2.shape[1]
    P.F, P.HID, P.DOUT = F, HID, DOUT

    src = np.asarray(edge_index[0], np.int64)
    dst = np.asarray(edge_index[1], np.int64)
    deg = np.bincount(dst, minlength=N).astype(np.float64) + 1.0
    dinv = (1.0 / np.sqrt(deg)).astype(np.float32)

    # src row mapping (NO self loops in the edge stream); two ranges by
    # local row: A = r < ASPL, B = r >= ASPL
    src_c = src // NPC
    src_r = src % NPC
    in_range_b = (src_r >= P.ASPL).astype(np.int64)
    src_local = np.where(in_range_b == 1,
                         src_c * P.BSPL + (src_r - P.ASPL),
                         src_c * P.ASPL + src_r)

    dst_core = dst // NPC
    dst_local = dst % NPC

    swglob = dst_local // SUB
    NSW = P.NSW
    key = in_range_b * NSW + swglob  # [E], in 0..2*NSW

    counts = np.zeros((NC, 2 * NSW), np.int64)
    for c in range(NC):
        m = dst_core == c
        counts[c] = np.bincount(key[m], minlength=2 * NSW)
    maxcnt = counts.max(axis=0)
    nchunks_key = (maxcnt + 127) // 128  # [2*NSW]

    P.plan = []
    for w in P.windows:
        per_s = []
        for s in range(2):
            bases = []
            for sw in range(w.sw0, w.sw0 + w.nsw):
                bases += [(sw - w.sw0) * SUB] * int(nchunks_key[s * NSW + sw])
            per_s.append(bases)
        P.plan.append(per_s)
    P.NCH = [[len(P.plan[wi][s]) for s in range(2)]
             for wi in range(len(P.windows))]
    P.TOTCH = sum(sum(n) for n in P.NCH)
    P.SLOTS = [sum(P.NCH[wi][s] for wi in range(len(P.windows)))
               * 128 for s in range(2)]

    key_off = np.zeros(2 * NSW, np.int64)
    key_choff = np.zeros(2 * NSW, np.int64)
    off_s = [0, 0]
    choff = 0
    for wi, w in enumerate(P.windows):
        for s in range(2):
            for sw in range(w.sw0, w.sw0 + w.nsw):
                k = s * NSW + sw
                key_off[k] = off_s[s]
                off_s[s] += int(nchunks_key[k]) * 128
                key_choff[k] = choff
                choff += int(nchunks_key[k])
    assert off_s[0] == P.SLOTS[0] and off_s[1] == P.SLOTS[1]
    assert choff == P.TOTCH

    # x~ (x * dinv) rows in the two-range layout [NA + NB, F], bf16
    xsc = x.astype(np.float32) * dinv[:, None]  # [N, F]
    xrows = np.zeros((P.NA + P.NB, F), np.float32)
    for c in range(NC):
        xrows[c * P.ASPL:(c + 1) * P.ASPL] = xsc[
            c * NPC: c * NPC + P.ASPL]
        nb_real = NPC - P.ASPL
        xrows[P.NA + c * P.BSPL: P.NA + c * P.BSPL + nb_real] = xsc[
            c * NPC + P.ASPL:(c + 1) * NPC]
    xrows = np.ascontiguousarray(xrows.astype(BF16))

    # x~ᵀ own block [F, BLK] for the self-loop term
    xT = np.zeros((F, BLK * NC), np.float32)
    for c in range(NC):
        xT[:, c * BLK: c * BLK + NPC] = xsc[c * NPC:(c + 1) * NPC].T
    xT = xT.astype(BF16)

    def wrap_idxs(flat):
        Sn = len(flat)
        assert Sn % 16 == 0
        w16 = flat.reshape(Sn // 16, 16).T  # [16, S/16]
        return np.tile(w16, (8, 1)).astype(np.int16)

    in_maps = []
    for c in range(NC):
        m = dst_core == c
        k_c = key[m]
        sl_c = src_local[m]
        dl_c = dst_local[m]
        order = np.argsort(k_c, kind="stable")
        k_c, sl_c, dl_c = k_c[order], sl_c[order], dl_c[order]
        cnt_c = np.bincount(k_c, minlength=2 * NSW)
        starts = np.zeros(2 * NSW, np.int64)
        starts[1:] = np.cumsum(cnt_c)[:-1]
        rank = np.arange(len(k_c)) - starts[k_c]
        slot = key_off[k_c] + rank
        s_of = (k_c >= NSW).astype(np.int64)

        idx_streams = []
        for s in range(2):
            st = np.zeros(P.SLOTS[s], np.int64)  # pad -> row 0 (S row is 0)
            ms = s_of == s
            st[slot[ms]] = sl_c[ms]
            idx_streams.append(st)

        Sm = np.zeros((P.TOTCH, 128, SUB), np.float32)
        ch_glob = key_choff[k_c] + rank // 128
        Sm[ch_glob, rank % 128, dl_c % SUB] = 1.0
        Sm = np.ascontiguousarray(Sm.transpose(1, 0, 2)).reshape(
            128, P.TOTCH * SUB).astype(BF16)

        dinvb = np.zeros((128, BLK), np.float32)
        dinvb[:, :NPC] = dinv[c * NPC:(c + 1) * NPC][None, :]
        ident = np.eye(128, dtype=np.float32)

        in_maps.append({
            "xrows": xrows,
            "xTown": np.ascontiguousarray(xT[:, c * BLK:(c + 1) * BLK]),
            "w1": W1.astype(np.float32).astype(BF16),
            "w2": W2.astype(np.float32).astype(BF16),
            "b1": b1.astype(np.float32).reshape(HID, 1),
            "b2": b2.astype(np.float32).reshape(DOUT, 1),
            "dinvb": dinvb.astype(BF16),
            "ident": ident.astype(BF16),
            "sall": Sm,
            "idxA": wrap_idxs(idx_streams[0]),
            "idxB": wrap_idxs(idx_streams[1]),
        })
    return in_maps


def postprocess(P, results):
    out = np.zeros((P.N, P.DOUT), np.float32)
    for c in range(P.NC):
        blk = results[c]["out"]  # [DOUT, BLK]
        out[c * P.NPC:(c + 1) * P.NPC] = blk[:, :P.NPC].T
    return out


# ---------------------------------------------------------------- builder
def build(P):
    import concourse.bacc as bacc
    import concourse.tile as tile
    import concourse.mybir as mybir

    dt = mybir.dt
    NC, BLK, SUB = P.NC, P.BLK, P.SUB
    F, HID, DOUT = P.F, P.HID, P.DOUT
    SA16 = P.SLOTS[0] // 16
    SB16 = P.SLOTS[1] // 16
    NQ = 4

    nc = bacc.Bacc("TRN2", target_bir_lowering=False, debug=False,
                   num_devices=NC, num_swdge_queues=NQ,
                   dynamic_dma_scratch_size=16384)
    xrows_d = nc.dram_tensor("xrows", [P.NA + P.NB, F], dt.bfloat16,
                             kind="ExternalInput")
    xTown_d = nc.dram_tensor("xTown", [F, BLK], dt.bfloat16,
                             kind="ExternalInput")
    w1_d = nc.dram_tensor("w1", [F, HID], dt.bfloat16, kind="ExternalInput")
    w2_d = nc.dram_tensor("w2", [HID, DOUT], dt.bfloat16,
                          kind="ExternalInput")
    b1_d = nc.dram_tensor("b1", [HID, 1], dt.float32, kind="ExternalInput")
    b2_d = nc.dram_tensor("b2", [DOUT, 1], dt.float32, kind="ExternalInput")
    dinvb_d = nc.dram_tensor("dinvb", [128, BLK], dt.bfloat16,
                             kind="ExternalInput")
    ident_d = nc.dram_tensor("ident", [128, 128], dt.bfloat16,
                             kind="ExternalInput")
    SCOLS = P.TOTCH * SUB
    MAXSW = max((P.NCH[wi][0] + P.NCH[wi][1]) * SUB
                for wi in range(len(P.windows)))
    sall_d = nc.dram_tensor("sall", [128, SCOLS], dt.bfloat16,
                            kind="ExternalInput")
    idxA_d = nc.dram_tensor("idxA", [128, SA16], dt.int16,
                            kind="ExternalInput")
    idxB_d = nc.dram_tensor("idxB", [128, SB16], dt.int16,
                            kind="ExternalInput")
    out_d = nc.dram_tensor("out", [DOUT, BLK], dt.float32,
                           kind="ExternalOutput")

    with tile.TileContext(nc) as tc:
        with (
            tc.tile_pool(name="dram", bufs=1, space="DRAM") as dram,
            tc.tile_pool(name="const", bufs=1) as cpool,
            tc.tile_pool(name="msgs", bufs=4) as mpool,
            tc.tile_pool(name="smat", bufs=2) as spool,
            tc.tile_pool(name="zxb", bufs=3) as zpool,
            tc.tile_pool(name="drain", bufs=3) as drpool,
            tc.tile_pool(name="rows", bufs=3) as rpool,
            tc.tile_pool(name="psum_z", bufs=3, space="PSUM") as pz,
            tc.tile_pool(name="psum_h", bufs=2, space="PSUM") as ph,
            tc.tile_pool(name="psum_tp", bufs=2, space="PSUM") as ptp,
            tc.tile_pool(name="psum_po", bufs=1, space="PSUM") as ppo,
        ):
            ag_in = dram.tile([BLK, HID], dt.bfloat16)
            table2a = dram.tile([P.NA, HID], dt.bfloat16,
                                addr_space="Shared")
            table2b = dram.tile([P.NB, HID], dt.bfloat16,
                                addr_space="Shared")

            # ---- constants to SBUF
            w1sb = cpool.tile([F, HID], dt.bfloat16)
            nc.sync.dma_start(w1sb[:], w1_d[:])
            w2sb = cpool.tile([HID, DOUT], dt.bfloat16)
            nc.sync.dma_start(w2sb[:], w2_d[:])
            b1sb = cpool.tile([HID, 1], dt.float32)
            nc.sync.dma_start(b1sb[:], b1_d[:])
            b2sb = cpool.tile([DOUT, 1], dt.float32)
            nc.sync.dma_start(b2sb[:], b2_d[:])
            dinvb = cpool.tile([128, BLK], dt.bfloat16)
            nc.sync.dma_start(dinvb[:], dinvb_d[:])
            ident = cpool.tile([128, 128], dt.bfloat16)
            nc.sync.dma_start(ident[:], ident_d[:])

            idxA = cpool.tile([128, SA16], dt.int16)
            nc.sync.dma_start(idxA[:], idxA_d[:])
            idxB = cpool.tile([128, SB16], dt.int16)
            nc.sync.dma_start(idxB[:], idxB_d[:])
            xts = cpool.tile([128, BLK], dt.bfloat16)
            nc.sync.dma_start(xts[:], xTown_d[:])
            h2b = cpool.tile([128, BLK], dt.bfloat16)

            def ag_range(lo, n, dst_tile):
                nc.gpsimd.collective_compute(
                    "AllGather",
                    mybir.AluOpType.bypass,
                    ins=[ag_in[lo:lo + n, :].opt()],
                    outs=[dst_tile[:].opt()],
                    replica_groups=[list(range(NC))],
                )

            # ---- flat job list over both layers: job = [layer, wi, s, nch, q]
            jobs = []
            qctr = 0
            for layer in (1, 2):
                for wi in range(len(P.windows)):
                    for s in (0, 1):
                        nch = P.NCH[wi][s]
                        if nch == 0:
                            continue
                        jobs.append([layer, wi, s, nch, qctr % NQ])
                        qctr += 1
            njobs = len(jobs)

            l1_ioffs = {}
            offs16 = [0, 0]
            for wi in range(len(P.windows)):
                for s in (0, 1):
                    nch = P.NCH[wi][s]
                    if nch == 0:
                        continue
                    l1_ioffs[(wi, s)] = offs16[s]
                    offs16[s] += nch * 8

            job_msgs = [None] * njobs

            def emit_gather(j):
                if job_msgs[j] is not None:
                    return
                layer, wi, s, nch, q = jobs[j]
                idx = idxA if s == 0 else idxB
                ioff = l1_ioffs[(wi, s)]
                if layer == 1:
                    tab = (xrows_d[0:P.NA, :] if s == 0
                           else xrows_d[P.NA:P.NA + P.NB, :])
                else:
                    tab = table2a[:] if s == 0 else table2b[:]
                mt = mpool.tile([128, nch, F], dt.bfloat16,
                                tag=f"m{s}", name=f"msgs_l{layer}_{wi}_{s}")
                job_msgs[j] = mt
                nc.gpsimd.dma_gather(
                    mt[:], tab,
                    idx[:, ioff: ioff + nch * 8],
                    nch * 128, nch * 128, F,
                    single_packet=False,
                    queue_num=q,
                )

            win_jobs = {}
            for j, (layer, wi, s, nch, q) in enumerate(jobs):
                win_jobs.setdefault((layer, wi), []).append(j)

            win_state = {}

            def emit_mms(layer, wi):
                w = P.windows[wi]
                cols = slice(w.col0, w.col0 + w.ncols)
                ncols = w.ncols
                nchA, nchB = P.NCH[wi]
                nch_tot = nchA + nchB
                soff = 0
                for wj in range(wi):
                    soff += P.NCH[wj][0] + P.NCH[wj][1]

                for j in win_jobs[(layer, wi)]:
                    emit_gather(j)

                zw = pz.tile([128, P.WIN], dt.float32, tag="z")
                src_self = xts if layer == 1 else h2b
                nc.tensor.matmul(
                    zw[:, :ncols], ident[:], src_self[:, cols],
                    start=True, stop=False,
                )

                swt = spool.tile([128, MAXSW], dt.bfloat16, tag="sw")
                nc.sync.dma_start(
                    swt[:, : nch_tot * SUB],
                    sall_d[:, soff * SUB:(soff + nch_tot) * SUB],
                )

                k_all = 0
                for s in (0, 1):
                    jlist = [j for j in win_jobs[(layer, wi)]
                             if jobs[j][2] == s]
                    if not jlist:
                        continue
                    mt = job_msgs[jlist[0]]
                    for k, base in enumerate(P.plan[wi][s]):
                        k_all += 1
                        nc.tensor.matmul(
                            zw[:, base:base + SUB],
                            mt[:, k, :],
                            swt[:, (k_all - 1) * SUB:k_all * SUB],
                            start=False, stop=(k_all == nch_tot),
                        )

                # free zw fast (off the PE stream) so the PSUM window pool
                # rotates; the rest of the tail is deferred one window.
                if layer == 1:
                    zxb = zpool.tile([128, P.WIN], dt.bfloat16, tag="zx")
                    nc.vector.tensor_copy(zxb[:, :ncols], zw[:, :ncols])
                    win_state[(layer, wi)] = zxb
                else:
                    qT = drpool.tile([128, P.WIN], dt.bfloat16, tag="qT")
                    nc.vector.tensor_tensor(
                        qT[:, :ncols], zw[:, :ncols], dinvb[:, cols],
                        op=mybir.AluOpType.mult,
                    )
                    win_state[(layer, wi)] = qT

            def emit_tail(layer, wi):
                w = P.windows[wi]
                cols = slice(w.col0, w.col0 + w.ncols)
                ncols = w.ncols
                if layer == 1:
                    zxb = win_state.pop((layer, wi))
                    z1 = ph.tile([128, P.WIN], dt.float32, tag="z1")
                    nc.tensor.matmul(
                        z1[:, :ncols], w1sb[:], zxb[:, :ncols],
                        start=True, stop=True,
                    )
                    t1 = drpool.tile([128, P.WIN], dt.float32, tag="t1")
                    nc.vector.tensor_tensor(
                        t1[:, :ncols], z1[:, :ncols], dinvb[:, cols],
                        op=mybir.AluOpType.mult,
                    )
                    t2 = drpool.tile([128, P.WIN], dt.float32, tag="t2")
                    nc.scalar.activation(
                        t2[:, :ncols], t1[:, :ncols],
                        mybir.ActivationFunctionType.Relu, bias=b1sb[:],
                    )
                    nc.vector.tensor_tensor(
                        h2b[:, cols], t2[:, :ncols], dinvb[:, cols],
                        op=mybir.AluOpType.mult,
                    )
                    for jc in range(0, ncols, 128):
                        nj = min(128, ncols - jc)
                        tp = ptp.tile([128, 128], dt.bfloat16, tag="tp")
                        nc.tensor.transpose(
                            tp[:nj, :],
                            h2b[:, w.col0 + jc: w.col0 + jc + nj],
                            ident[:]
                        )
                        hr = rpool.tile([128, 128], dt.bfloat16,
                                        tag="hr")
                        if (jc // 128) % 2 == 0:
                            nc.vector.tensor_copy(hr[:nj, :], tp[:nj, :])
                        else:
                            nc.scalar.copy(hr[:nj, :], tp[:nj, :])
                        nc.sync.dma_start(
                            ag_in[w.col0 + jc: w.col0 + jc + nj, :],
                            hr[:nj, :]
                        )
                else:
                    qT = win_state.pop((layer, wi))
                    po = ppo.tile([DOUT, P.WIN], dt.float32, tag="po")
                    nc.tensor.matmul(
                        po[:, :ncols], w2sb[:], qT[:, :ncols],
                        start=True, stop=True,
                    )
                    ot = rpool.tile([DOUT, P.WIN], dt.float32, tag="ot")
                    nc.scalar.activation(
                        ot[:, :ncols], po[:, :ncols],
                        mybir.ActivationFunctionType.Relu, bias=b2sb[:],
                    )
                    nc.sync.dma_start(out_d[:, cols], ot[:, :ncols])

            nwin = len(P.windows)
            pending = None
            for wi in range(nwin):
                emit_mms(1, wi)
                if pending is not None:
                    emit_tail(1, pending)
                    if pending == P.AG_SPLIT_WIN - 1:
                        ag_range(0, P.ASPL, table2a)
                pending = wi
            emit_tail(1, pending)
            # keep the gather pipeline busy during AG-B: the first four
            # layer-2 range-A gathers only need table2a (already gathered)
            for j in win_jobs.get((2, 0), []) + win_jobs.get((2, 1), []) \
                    + win_jobs.get((2, 2), []) + win_jobs.get((2, 3), []):
                if jobs[j][2] == 0:
                    emit_gather(j)
            ag_range(P.ASPL, P.BSPL, table2b)
            pending = None
            for wi in range(nwin):
                emit_mms(2, wi)
                if pending is not None:
                    emit_tail(2, pending)
                pending = wi
            emit_tail(2, pending)

    nc.compile()
    return nc


# ----------------------------------------------------------------- kernel()
_BUILD_CACHE = {}
_LAST = {}


def _get_nc(P, key, **bkw):
    ent = _BUILD_CACHE.get(key)
    if ent is None:
        ent = build(P, **bkw)
        _BUILD_CACHE[key] = ent
    return ent


def kernel(x, edge_index, W1, b1, W2, b2):
    import numpy as np
    x = np.asarray(x)
    edge_index = np.asarray(edge_index)
    N = x.shape[0]
    NC = 8
    P = make_structure(N, NC)
    in_maps = prep(P, x, edge_index, np.asarray(W1), np.asarray(b1),
                   np.asarray(W2), np.asarray(b2))
    key = (N, x.shape[1], np.asarray(W2).shape[1], P.TOTCH,
           tuple(tuple(n) for n in P.NCH))
    nc = _get_nc(P, key)
    _LAST.update(P=P, in_maps=in_maps, nc=nc)
    from concourse.bass_utils import run_bass_kernel_spmd
    res = run_bass_kernel_spmd(nc, in_maps, core_ids=list(range(NC)))
    return postprocess(P, res.results).astype(np.float32)


# revision 14
# speedup vs baseline: 1.0497x; 1.0497x over previous
"""GCN 2-layer kernel for TRN2 x8 cores — host prep + Bass/Tile builder.

Math: out1 = relu(dinv ⊙ (Aᵀ (dinv ⊙ x)) @ W1 + b1)
      out2 = relu(dinv ⊙ (Aᵀ (dinv ⊙ out1)) @ W2 + b2)
with A = adjacency incl. self-loops, dinv = rsqrt(in-degree incl self).
(W1/W2 commute with the segment-sum, so both are applied AFTER the
per-window aggregation — layer 1 gathers raw x~ rows, not x~@W1.)

Device plan (SPMD, 8 cores, one program), v9:
- nodes dst-sharded by core (NPC per core, BLK = padded block).
- src nodes split into two ranges BY LOCAL ROW, not by core:
  range A = local rows [0, ASPL), range B = [ASPL, BLK). Table row =
  c*ASPL + r (A) or c*BSPL + (r - ASPL) (B). Both ranges stay int16-
  addressable, and each range's layer-2 table is a SEPARATE Shared
  tensor filled by its own AllGather: AG-A fires as soon as layer-1
  windows covering rows < ASPL are done (hidden under the remaining
  windows); AG-B fires at layer-1 end (only ~35us exposed).
- layer-1 gather source = xrows (replicated input, the same two-range
  row layout). No device-side table build — gathers start at ~10us.
- edges bucketed by (src range, dst 128-col sub-window); chunks of
  128 edges; chunk counts shared across cores (max over cores); pad
  slots gather row 0 with a zero one-hot row.
- messages fetched with dma_gather (bf16 256B rows) round-robin over
  4 SWDGE queues (descriptor generation on Q7 core pairs is the
  bottleneck). msgs pool 4 deep per range so the message-tile WAR
  doesn't gate gather issue.
- one-hot S streamed from DRAM; segment-sum via PE (msgs stationary,
  S moving, PSUM window accumulation); self-loop opens each window.
- layer1 tail: zxb=bf16(zx); z1=W1ᵀ@zxb; h2b=dinv*relu(dinv*z1+b1);
  transpose -> ag_in slab. layer2 tail: out = relu(W2ᵀ(dinv*zh)+b2).
"""
import numpy as np
import ml_dtypes

BF16 = ml_dtypes.bfloat16


# ---------------------------------------------------------------- structure
class Struct:
    pass


def make_structure(N, NC, WIN=512, SUB=128):
    P = Struct()
    P.N, P.NC, P.WIN, P.SUB = N, NC, WIN, SUB
    assert N % NC == 0
    P.NPC = N // NC
    P.BLK = ((P.NPC + 1 + 31) // 32) * 32
    assert P.BLK % SUB == 0 and P.BLK % 128 == 0
    P.windows = []
    col0 = 0
    while col0 < P.BLK:
        ncols = min(WIN, P.BLK - col0)
        w = Struct()
        w.col0, w.ncols = col0, ncols
        w.sw0, w.nsw = col0 // SUB, ncols // SUB
        P.windows.append(w)
        col0 += ncols
    # split source rows at the latest window boundary that keeps the
    # range-A table int16-addressable (min exposure for the second AG)
    P.AG_SPLIT_WIN = max(wi for wi in range(1, len(P.windows))
                         if NC * P.windows[wi].col0 <= 32768)
    P.ASPL = P.windows[P.AG_SPLIT_WIN].col0  # local rows in range A
    P.BSPL = P.BLK - P.ASPL
    P.NA = NC * P.ASPL  # total range-A table rows
    P.NB = NC * P.BSPL
    assert P.NA <= 32768 and P.NB <= 32768
    P.NSW = P.BLK // SUB
    return P


# ---------------------------------------------------------------- host prep
def prep(P, x, edge_index, W1, b1, W2, b2):
    """Returns in_maps — the per-core input dict list. Also fills P.plan."""
    N, NC, NPC, BLK, SUB = P.N, P.NC, P.NPC, P.BLK, P.SUB
    F = x.shape[1]
    HID = W1.shape[1]
    DOUT = W2.shape[1]
    P.F, P.HID, P.DOUT = F, HID, DOUT

    src = np.asarray(edge_index[0], np.int64)
    dst = np.asarray(edge_index[1], np.int64)
    deg = np.bincount(dst, minlength=N).astype(np.float64) + 1.0
    dinv = (1.0 / np.sqrt(deg)).astype(np.float32)

    # src row mapping (NO self loops in the edge stream); two ranges by
    # local row: A = r < ASPL, B = r >= ASPL
    src_c = src // NPC
    src_r = src % NPC
    in_range_b = (src_r >= P.ASPL).astype(np.int64)
    src_local = np.where(in_range_b == 1,
                         src_c * P.BSPL + (src_r - P.ASPL),
                         src_c * P.ASPL + src_r)

    dst_core = dst // NPC
    dst_local = dst % NPC

    swglob = dst_local // SUB
    NSW = P.NSW
    key = in_range_b * NSW + swglob  # [E], in 0..2*NSW

    counts = np.zeros((NC, 2 * NSW), np.int64)
    for c in range(NC):
        m = dst_core == c
        counts[c] = np.bincount(key[m], minlength=2 * NSW)
    maxcnt = counts.max(axis=0)
    nchunks_key = (maxcnt + 127) // 128  # [2*NSW]

    P.plan = []
    for w in P.windows:
        per_s = []
        for s in range(2):
            bases = []
            for sw in range(w.sw0, w.sw0 + w.nsw):
                bases += [(sw - w.sw0) * SUB] * int(nchunks_key[s * NSW + sw])
            per_s.append(bases)
        P.plan.append(per_s)
    P.NCH = [[len(P.plan[wi][s]) for s in range(2)]
             for wi in range(len(P.windows))]
    P.TOTCH = sum(sum(n) for n in P.NCH)
    P.SLOTS = [sum(P.NCH[wi][s] for wi in range(len(P.windows)))
               * 128 for s in range(2)]

    key_off = np.zeros(2 * NSW, np.int64)
    key_choff = np.zeros(2 * NSW, np.int64)
    off_s = [0, 0]
    choff = 0
    for wi, w in enumerate(P.windows):
        for s in range(2):
            for sw in range(w.sw0, w.sw0 + w.nsw):
                k = s * NSW + sw
                key_off[k] = off_s[s]
                off_s[s] += int(nchunks_key[k]) * 128
                key_choff[k] = choff
                choff += int(nchunks_key[k])
    assert off_s[0] == P.SLOTS[0] and off_s[1] == P.SLOTS[1]
    assert choff == P.TOTCH

    # x~ (x * dinv) rows in the two-range layout [NA + NB, F], bf16
    xsc = x.astype(np.float32) * dinv[:, None]  # [N, F]
    xrows = np.zeros((P.NA + P.NB, F), np.float32)
    for c in range(NC):
        xrows[c * P.ASPL:(c + 1) * P.ASPL] = xsc[
            c * NPC: c * NPC + P.ASPL]
        nb_real = NPC - P.ASPL
        xrows[P.NA + c * P.BSPL: P.NA + c * P.BSPL + nb_real] = xsc[
            c * NPC + P.ASPL:(c + 1) * NPC]
    xrows = np.ascontiguousarray(xrows.astype(BF16))

    # x~ᵀ own block [F, BLK] for the self-loop term
    xT = np.zeros((F, BLK * NC), np.float32)
    for c in range(NC):
        xT[:, c * BLK: c * BLK + NPC] = xsc[c * NPC:(c + 1) * NPC].T
    xT = xT.astype(BF16)

    def wrap_idxs(flat):
        Sn = len(flat)
        assert Sn % 16 == 0
        w16 = flat.reshape(Sn // 16, 16).T  # [16, S/16]
        return np.tile(w16, (8, 1)).astype(np.int16)

    in_maps = []
    for c in range(NC):
        m = dst_core == c
        k_c = key[m]
        sl_c = src_local[m]
        dl_c = dst_local[m]
        order = np.argsort(k_c, kind="stable")
        k_c, sl_c, dl_c = k_c[order], sl_c[order], dl_c[order]
        cnt_c = np.bincount(k_c, minlength=2 * NSW)
        starts = np.zeros(2 * NSW, np.int64)
        starts[1:] = np.cumsum(cnt_c)[:-1]
        rank = np.arange(len(k_c)) - starts[k_c]
        slot = key_off[k_c] + rank
        s_of = (k_c >= NSW).astype(np.int64)

        idx_streams = []
        for s in range(2):
            st = np.zeros(P.SLOTS[s], np.int64)  # pad -> row 0 (S row is 0)
            ms = s_of == s
            st[slot[ms]] = sl_c[ms]
            idx_streams.append(st)

        Sm = np.zeros((P.TOTCH, 128, SUB), np.float32)
        ch_glob = key_choff[k_c] + rank // 128
        Sm[ch_glob, rank % 128, dl_c % SUB] = 1.0
        Sm = np.ascontiguousarray(Sm.transpose(1, 0, 2)).reshape(
            128, P.TOTCH * SUB).astype(BF16)

        dinvb = np.zeros((128, BLK), np.float32)
        dinvb[:, :NPC] = dinv[c * NPC:(c + 1) * NPC][None, :]
        ident = np.eye(128, dtype=np.float32)

        in_maps.append({
            "xrows": xrows,
            "xTown": np.ascontiguousarray(xT[:, c * BLK:(c + 1) * BLK]),
            "w1": W1.astype(np.float32).astype(BF16),
            "w2": W2.astype(np.float32).astype(BF16),
            "b1": b1.astype(np.float32).reshape(HID, 1),
            "b2": b2.astype(np.float32).reshape(DOUT, 1),
            "dinvb": dinvb.astype(BF16),
            "ident": ident.astype(BF16),
            "sall": Sm,
            "idxA": wrap_idxs(idx_streams[0]),
            "idxB": wrap_idxs(idx_streams[1]),
        })
    return in_maps


def postprocess(P, results):
    out = np.zeros((P.N, P.DOUT), np.float32)
    for c in range(P.NC):
        blk = results[c]["out"]  # [DOUT, BLK]
        out[c * P.NPC:(c + 1) * P.NPC] = blk[:, :P.NPC].T
    return out


# ---------------------------------------------------------------- builder
def build(P):
    import concourse.bacc as bacc
    import concourse.tile as tile
    import concourse.mybir as mybir

    dt = mybir.dt
    NC, BLK, SUB = P.NC, P.BLK, P.SUB
    F, HID, DOUT = P.F, P.HID, P.DOUT
    SA16 = P.SLOTS[0] // 16
    SB16 = P.SLOTS[1] // 16
    NQ = 4

    nc = bacc.Bacc("TRN2", target_bir_lowering=False, debug=False,
                   num_devices=NC, num_swdge_queues=NQ,
                   dynamic_dma_scratch_size=12288)
    xrows_d = nc.dram_tensor("xrows", [P.NA + P.NB, F], dt.bfloat16,
                             kind="ExternalInput")
    xTown_d = nc.dram_tensor("xTown", [F, BLK], dt.bfloat16,
                             kind="ExternalInput")
    w1_d = nc.dram_tensor("w1", [F, HID], dt.bfloat16, kind="ExternalInput")
    w2_d = nc.dram_tensor("w2", [HID, DOUT], dt.bfloat16,
                          kind="ExternalInput")
    b1_d = nc.dram_tensor("b1", [HID, 1], dt.float32, kind="ExternalInput")
    b2_d = nc.dram_tensor("b2", [DOUT, 1], dt.float32, kind="ExternalInput")
    dinvb_d = nc.dram_tensor("dinvb", [128, BLK], dt.bfloat16,
                             kind="ExternalInput")
    ident_d = nc.dram_tensor("ident", [128, 128], dt.bfloat16,
                             kind="ExternalInput")
    SCOLS = P.TOTCH * SUB
    MAXSW = max((P.NCH[wi][0] + P.NCH[wi][1]) * SUB
                for wi in range(len(P.windows)))
    sall_d = nc.dram_tensor("sall", [128, SCOLS], dt.bfloat16,
                            kind="ExternalInput")
    idxA_d = nc.dram_tensor("idxA", [128, SA16], dt.int16,
                            kind="ExternalInput")
    idxB_d = nc.dram_tensor("idxB", [128, SB16], dt.int16,
                            kind="ExternalInput")
    out_d = nc.dram_tensor("out", [DOUT, BLK], dt.float32,
                           kind="ExternalOutput")

    with tile.TileContext(nc) as tc:
        with (
            tc.tile_pool(name="dram", bufs=1, space="DRAM") as dram,
            tc.tile_pool(name="const", bufs=1) as cpool,
            tc.tile_pool(name="msgs", bufs=5) as mpool,
            tc.tile_pool(name="smat", bufs=2) as spool,
            tc.tile_pool(name="zxb", bufs=3) as zpool,
            tc.tile_pool(name="drain", bufs=3) as drpool,
            tc.tile_pool(name="rows", bufs=3) as rpool,
            tc.tile_pool(name="psum_z", bufs=3, space="PSUM") as pz,
            tc.tile_pool(name="psum_h", bufs=2, space="PSUM") as ph,
            tc.tile_pool(name="psum_tp", bufs=2, space="PSUM") as ptp,
            tc.tile_pool(name="psum_po", bufs=1, space="PSUM") as ppo,
        ):
            ag_in = dram.tile([BLK, HID], dt.bfloat16)
            table2a = dram.tile([P.NA, HID], dt.bfloat16,
                                addr_space="Shared")
            table2b = dram.tile([P.NB, HID], dt.bfloat16,
                                addr_space="Shared")

            # ---- constants to SBUF
            w1sb = cpool.tile([F, HID], dt.bfloat16)
            nc.sync.dma_start(w1sb[:], w1_d[:])
            w2sb = cpool.tile([HID, DOUT], dt.bfloat16)
            nc.sync.dma_start(w2sb[:], w2_d[:])
            b1sb = cpool.tile([HID, 1], dt.float32)
            nc.sync.dma_start(b1sb[:], b1_d[:])
            b2sb = cpool.tile([DOUT, 1], dt.float32)
            nc.sync.dma_start(b2sb[:], b2_d[:])
            dinvb = cpool.tile([128, BLK], dt.bfloat16)
            nc.sync.dma_start(dinvb[:], dinvb_d[:])
            ident = cpool.tile([128, 128], dt.bfloat16)
            nc.sync.dma_start(ident[:], ident_d[:])

            idxA = cpool.tile([128, SA16], dt.int16)
            nc.sync.dma_start(idxA[:], idxA_d[:])
            idxB = cpool.tile([128, SB16], dt.int16)
            nc.sync.dma_start(idxB[:], idxB_d[:])
            xts = cpool.tile([128, BLK], dt.bfloat16)
            nc.sync.dma_start(xts[:], xTown_d[:])
            h2b = cpool.tile([128, BLK], dt.bfloat16)

            def ag_range(lo, n, dst_tile):
                nc.gpsimd.collective_compute(
                    "AllGather",
                    mybir.AluOpType.bypass,
                    ins=[ag_in[lo:lo + n, :].opt()],
                    outs=[dst_tile[:].opt()],
                    replica_groups=[list(range(NC))],
                )

            # ---- flat job list over both layers: job = [layer, wi, s, nch, q]
            jobs = []
            qctr = 0
            for layer in (1, 2):
                for wi in range(len(P.windows)):
                    for s in (0, 1):
                        nch = P.NCH[wi][s]
                        if nch == 0:
                            continue
                        jobs.append([layer, wi, s, nch, qctr % NQ])
                        qctr += 1
            njobs = len(jobs)

            l1_ioffs = {}
            offs16 = [0, 0]
            for wi in range(len(P.windows)):
                for s in (0, 1):
                    nch = P.NCH[wi][s]
                    if nch == 0:
                        continue
                    l1_ioffs[(wi, s)] = offs16[s]
                    offs16[s] += nch * 8

            job_msgs = [None] * njobs

            def emit_gather(j):
                if job_msgs[j] is not None:
                    return
                layer, wi, s, nch, q = jobs[j]
                idx = idxA if s == 0 else idxB
                ioff = l1_ioffs[(wi, s)]
                if layer == 1:
                    tab = (xrows_d[0:P.NA, :] if s == 0
                           else xrows_d[P.NA:P.NA + P.NB, :])
                else:
                    tab = table2a[:] if s == 0 else table2b[:]
                mt = mpool.tile([128, nch, F], dt.bfloat16,
                                tag=f"m{s}", name=f"msgs_l{layer}_{wi}_{s}")
                job_msgs[j] = mt
                nc.gpsimd.dma_gather(
                    mt[:], tab,
                    idx[:, ioff: ioff + nch * 8],
                    nch * 128, nch * 128, F,
                    single_packet=False,
                    queue_num=q,
                )

            win_jobs = {}
            for j, (layer, wi, s, nch, q) in enumerate(jobs):
                win_jobs.setdefault((layer, wi), []).append(j)

            win_state = {}

            def emit_mms(layer, wi):
                w = P.windows[wi]
                cols = slice(w.col0, w.col0 + w.ncols)
                ncols = w.ncols
                nchA, nchB = P.NCH[wi]
                nch_tot = nchA + nchB
                soff = 0
                for wj in range(wi):
                    soff += P.NCH[wj][0] + P.NCH[wj][1]

                for j in win_jobs[(layer, wi)]:
                    emit_gather(j)

                zw = pz.tile([128, P.WIN], dt.float32, tag="z")
                src_self = xts if layer == 1 else h2b
                nc.tensor.matmul(
                    zw[:, :ncols], ident[:], src_self[:, cols],
                    start=True, stop=False,
                )

                swt = spool.tile([128, MAXSW], dt.bfloat16, tag="sw")
                nc.sync.dma_start(
                    swt[:, : nch_tot * SUB],
                    sall_d[:, soff * SUB:(soff + nch_tot) * SUB],
                )

                k_all = 0
                for s in (0, 1):
                    jlist = [j for j in win_jobs[(layer, wi)]
                             if jobs[j][2] == s]
                    if not jlist:
                        continue
                    mt = job_msgs[jlist[0]]
                    for k, base in enumerate(P.plan[wi][s]):
                        k_all += 1
                        nc.tensor.matmul(
                            zw[:, base:base + SUB],
                            mt[:, k, :],
                            swt[:, (k_all - 1) * SUB:k_all * SUB],
                            start=False, stop=(k_all == nch_tot),
                        )

                # free zw fast (off the PE stream) so the PSUM window pool
                # rotates; the rest of the tail is deferred one window.
                if layer == 1:
                    zxb = zpool.tile([128, P.WIN], dt.bfloat16, tag="zx")
                    nc.vector.tensor_copy(zxb[:, :ncols], zw[:, :ncols])
                    win_state[(layer, wi)] = zxb
                else:
                    qT = drpool.tile([128, P.WIN], dt.bfloat16, tag="qT")
                    nc.vector.tensor_tensor(
                        qT[:, :ncols], zw[:, :ncols], dinvb[:, cols],
                        op=mybir.AluOpType.mult,
                    )
                    po = ppo.tile([DOUT, P.WIN], dt.float32, tag="po")
                    nc.tensor.matmul(
                        po[:, :ncols], w2sb[:], qT[:, :ncols],
                        start=True, stop=True,
                    )
                    ot = rpool.tile([DOUT, P.WIN], dt.float32, tag="ot")
                    nc.scalar.activation(
                        ot[:, :ncols], po[:, :ncols],
                        mybir.ActivationFunctionType.Relu, bias=b2sb[:],
                    )
                    nc.sync.dma_start(out_d[:, cols], ot[:, :ncols])

            def emit_tail(layer, wi):
                w = P.windows[wi]
                cols = slice(w.col0, w.col0 + w.ncols)
                ncols = w.ncols
                if layer == 1:
                    zxb = win_state.pop((layer, wi))
                    z1 = ph.tile([128, P.WIN], dt.float32, tag="z1")
                    nc.tensor.matmul(
                        z1[:, :ncols], w1sb[:], zxb[:, :ncols],
                        start=True, stop=True,
                    )
                    t1 = drpool.tile([128, P.WIN], dt.float32, tag="t1")
                    nc.vector.tensor_tensor(
                        t1[:, :ncols], z1[:, :ncols], dinvb[:, cols],
                        op=mybir.AluOpType.mult,
                    )
                    t2 = drpool.tile([128, P.WIN], dt.float32, tag="t2")
                    nc.scalar.activation(
                        t2[:, :ncols], t1[:, :ncols],
                        mybir.ActivationFunctionType.Relu, bias=b1sb[:],
                    )
                    nc.vector.tensor_tensor(
                        h2b[:, cols], t2[:, :ncols], dinvb[:, cols],
                        op=mybir.AluOpType.mult,
                    )
                    for jc in range(0, ncols, 128):
                        nj = min(128, ncols - jc)
                        tp = ptp.tile([128, 128], dt.bfloat16, tag="tp")
                        nc.tensor.transpose(
                            tp[:nj, :],
                            h2b[:, w.col0 + jc: w.col0 + jc + nj],
                            ident[:]
                        )
                        hr = rpool.tile([128, 128], dt.bfloat16,
                                        tag="hr")
                        if (jc // 128) % 2 == 0:
                            nc.vector.tensor_copy(hr[:nj, :], tp[:nj, :])
                        else:
                            nc.scalar.copy(hr[:nj, :], tp[:nj, :])
                        nc.sync.dma_start(
                            ag_in[w.col0 + jc: w.col0 + jc + nj, :],
                            hr[:nj, :]
                        )

            nwin = len(P.windows)
            pending = None
            for wi in range(nwin):
                emit_mms(1, wi)
                if pending is not None:
                    emit_tail(1, pending)
                    if pending == P.AG_SPLIT_WIN - 1:
                        ag_range(0, P.ASPL, table2a)
                pending = wi
            emit_tail(1, pending)
            ag_range(P.ASPL, P.BSPL, table2b)
            for wi in range(nwin):
                emit_mms(2, wi)

    nc.compile()
    return nc


# ----------------------------------------------------------------- kernel()
_BUILD_CACHE = {}
_LAST = {}


def _get_nc(P, key, **bkw):
    ent = _BUILD_CACHE.get(key)
    if ent is None:
        ent = build(P, **bkw)
        _BUILD_CACHE[key] = ent
    return ent


def kernel(x, edge_index, W1, b1, W2, b2):
    import numpy as np
    x = np.asarray(x)
    edge_index = np.asarray(edge_index)
    N = x.shape[0]
    NC = 8
    P = make_structure(N, NC)
    in_maps = prep(P, x, edge_index, np.asarray(W1), np.asarray(b1),
                   np.asarray(W2), np.asarray(b2))
    key = (N, x.shape[1], np.asarray(W2).shape[1], P.TOTCH,
           tuple(tuple(n) for n in P.NCH))
    nc = _get_nc(P, key)
    _LAST.update(P=P, in_maps=in_maps, nc=nc)
    from concourse.bass_utils import run_bass_kernel_spmd
    res = run_bass_kernel_spmd(nc, in_maps, core_ids=list(range(NC)))
    return postprocess(P, res.results).astype(np.float32)


# revision 15
# speedup vs baseline: 1.1066x; 1.0542x over previous
"""GCN 2-layer kernel for TRN2 x8 cores — host prep + Bass/Tile builder.

Math: out1 = relu(dinv ⊙ (Aᵀ (dinv ⊙ x)) @ W1 + b1)
      out2 = relu(dinv ⊙ (Aᵀ (dinv ⊙ out1)) @ W2 + b2)
with A = adjacency incl. self-loops, dinv = rsqrt(in-degree incl self).
(W1/W2 commute with the segment-sum, so both are applied AFTER the
per-window aggregation — layer 1 gathers raw x~ rows, not x~@W1.)

Device plan (SPMD, 8 cores, one program), v9:
- nodes dst-sharded by core (NPC per core, BLK = padded block).
- src nodes split into two ranges BY LOCAL ROW, not by core:
  range A = local rows [0, ASPL), range B = [ASPL, BLK). Table row =
  c*ASPL + r (A) or c*BSPL + (r - ASPL) (B). Both ranges stay int16-
  addressable, and each range's layer-2 table is a SEPARATE Shared
  tensor filled by its own AllGather: AG-A fires as soon as layer-1
  windows covering rows < ASPL are done (hidden under the remaining
  windows); AG-B fires at layer-1 end (only ~35us exposed).
- layer-1 gather source = xrows (replicated input, the same two-range
  row layout). No device-side table build — gathers start at ~10us.
- edges bucketed by (src range, dst 128-col sub-window); chunks of
  128 edges; chunk counts shared across cores (max over cores); pad
  slots gather row 0 with a zero one-hot row.
- messages fetched with dma_gather (bf16 256B rows) round-robin over
  4 SWDGE queues (descriptor generation on Q7 core pairs is the
  bottleneck). msgs pool 4 deep per range so the message-tile WAR
  doesn't gate gather issue.
- one-hot S streamed from DRAM; segment-sum via PE (msgs stationary,
  S moving, PSUM window accumulation); self-loop opens each window.
- layer1 tail: zxb=bf16(zx); z1=W1ᵀ@zxb; h2b=dinv*relu(dinv*z1+b1);
  transpose -> ag_in slab. layer2 tail: out = relu(W2ᵀ(dinv*zh)+b2).
"""
import numpy as np
import ml_dtypes

BF16 = ml_dtypes.bfloat16


# ---------------------------------------------------------------- structure
class Struct:
    pass


def make_structure(N, NC, WIN=512, SUB=128):
    P = Struct()
    P.N, P.NC, P.WIN, P.SUB = N, NC, WIN, SUB
    assert N % NC == 0
    P.NPC = N // NC
    P.BLK = ((P.NPC + 1 + 31) // 32) * 32
    assert P.BLK % SUB == 0 and P.BLK % 128 == 0
    P.windows = []
    col0 = 0
    while col0 < P.BLK:
        ncols = min(WIN, P.BLK - col0)
        w = Struct()
        w.col0, w.ncols = col0, ncols
        w.sw0, w.nsw = col0 // SUB, ncols // SUB
        P.windows.append(w)
        col0 += ncols
    # split source rows at a window boundary near BLK/2 (balanced ranges
    # pipeline the gather queues best; a later split imbalances job sizes)
    P.AG_SPLIT_WIN = len(P.windows) // 2  # windows [0, this) are range A
    P.ASPL = P.windows[P.AG_SPLIT_WIN].col0  # local rows in range A
    P.BSPL = P.BLK - P.ASPL
    P.NA = NC * P.ASPL  # total range-A table rows
    P.NB = NC * P.BSPL
    assert P.NA <= 32768 and P.NB <= 32768
    P.NSW = P.BLK // SUB
    return P


# ---------------------------------------------------------------- host prep
def prep(P, x, edge_index, W1, b1, W2, b2):
    """Returns in_maps — the per-core input dict list. Also fills P.plan."""
    N, NC, NPC, BLK, SUB = P.N, P.NC, P.NPC, P.BLK, P.SUB
    F = x.shape[1]
    HID = W1.shape[1]
    DOUT = W2.shape[1]
    P.F, P.HID, P.DOUT = F, HID, DOUT

    src = np.asarray(edge_index[0], np.int64)
    dst = np.asarray(edge_index[1], np.int64)
    deg = np.bincount(dst, minlength=N).astype(np.float64) + 1.0
    dinv = (1.0 / np.sqrt(deg)).astype(np.float32)

    # src row mapping (NO self loops in the edge stream); two ranges by
    # local row: A = r < ASPL, B = r >= ASPL
    src_c = src // NPC
    src_r = src % NPC
    in_range_b = (src_r >= P.ASPL).astype(np.int64)
    src_local = np.where(in_range_b == 1,
                         src_c * P.BSPL + (src_r - P.ASPL),
                         src_c * P.ASPL + src_r)

    dst_core = dst // NPC
    dst_local = dst % NPC

    swglob = dst_local // SUB
    NSW = P.NSW
    key = in_range_b * NSW + swglob  # [E], in 0..2*NSW

    counts = np.zeros((NC, 2 * NSW), np.int64)
    for c in range(NC):
        m = dst_core == c
        counts[c] = np.bincount(key[m], minlength=2 * NSW)
    maxcnt = counts.max(axis=0)
    nchunks_key = (maxcnt + 127) // 128  # [2*NSW]

    P.plan = []
    for w in P.windows:
        per_s = []
        for s in range(2):
            bases = []
            for sw in range(w.sw0, w.sw0 + w.nsw):
                bases += [(sw - w.sw0) * SUB] * int(nchunks_key[s * NSW + sw])
            per_s.append(bases)
        P.plan.append(per_s)
    P.NCH = [[len(P.plan[wi][s]) for s in range(2)]
             for wi in range(len(P.windows))]
    P.TOTCH = sum(sum(n) for n in P.NCH)
    P.SLOTS = [sum(P.NCH[wi][s] for wi in range(len(P.windows)))
               * 128 for s in range(2)]

    key_off = np.zeros(2 * NSW, np.int64)
    key_choff = np.zeros(2 * NSW, np.int64)
    off_s = [0, 0]
    choff = 0
    for wi, w in enumerate(P.windows):
        for s in range(2):
            for sw in range(w.sw0, w.sw0 + w.nsw):
                k = s * NSW + sw
                key_off[k] = off_s[s]
                off_s[s] += int(nchunks_key[k]) * 128
                key_choff[k] = choff
                choff += int(nchunks_key[k])
    assert off_s[0] == P.SLOTS[0] and off_s[1] == P.SLOTS[1]
    assert choff == P.TOTCH

    # x~ (x * dinv) rows in the two-range layout [NA + NB, F], bf16
    xsc = x.astype(np.float32) * dinv[:, None]  # [N, F]
    xrows = np.zeros((P.NA + P.NB, F), np.float32)
    for c in range(NC):
        xrows[c * P.ASPL:(c + 1) * P.ASPL] = xsc[
            c * NPC: c * NPC + P.ASPL]
        nb_real = NPC - P.ASPL
        xrows[P.NA + c * P.BSPL: P.NA + c * P.BSPL + nb_real] = xsc[
            c * NPC + P.ASPL:(c + 1) * NPC]
    xrows = np.ascontiguousarray(xrows.astype(BF16))

    # x~ᵀ own block [F, BLK] for the self-loop term
    xT = np.zeros((F, BLK * NC), np.float32)
    for c in range(NC):
        xT[:, c * BLK: c * BLK + NPC] = xsc[c * NPC:(c + 1) * NPC].T
    xT = xT.astype(BF16)

    def wrap_idxs(flat):
        Sn = len(flat)
        assert Sn % 16 == 0
        w16 = flat.reshape(Sn // 16, 16).T  # [16, S/16]
        return np.tile(w16, (8, 1)).astype(np.int16)

    in_maps = []
    for c in range(NC):
        m = dst_core == c
        k_c = key[m]
        sl_c = src_local[m]
        dl_c = dst_local[m]
        order = np.argsort(k_c, kind="stable")
        k_c, sl_c, dl_c = k_c[order], sl_c[order], dl_c[order]
        cnt_c = np.bincount(k_c, minlength=2 * NSW)
        starts = np.zeros(2 * NSW, np.int64)
        starts[1:] = np.cumsum(cnt_c)[:-1]
        rank = np.arange(len(k_c)) - starts[k_c]
        slot = key_off[k_c] + rank
        s_of = (k_c >= NSW).astype(np.int64)

        idx_streams = []
        for s in range(2):
            st = np.zeros(P.SLOTS[s], np.int64)  # pad -> row 0 (S row is 0)
            ms = s_of == s
            st[slot[ms]] = sl_c[ms]
            idx_streams.append(st)

        Sm = np.zeros((P.TOTCH, 128, SUB), np.float32)
        ch_glob = key_choff[k_c] + rank // 128
        Sm[ch_glob, rank % 128, dl_c % SUB] = 1.0
        Sm = np.ascontiguousarray(Sm.transpose(1, 0, 2)).reshape(
            128, P.TOTCH * SUB).astype(BF16)

        dinvb = np.zeros((128, BLK), np.float32)
        dinvb[:, :NPC] = dinv[c * NPC:(c + 1) * NPC][None, :]
        ident = np.eye(128, dtype=np.float32)

        in_maps.append({
            "xrows": xrows,
            "xTown": np.ascontiguousarray(xT[:, c * BLK:(c + 1) * BLK]),
            "w1": W1.astype(np.float32).astype(BF16),
            "w2": W2.astype(np.float32).astype(BF16),
            "b1": b1.astype(np.float32).reshape(HID, 1),
            "b2": b2.astype(np.float32).reshape(DOUT, 1),
            "dinvb": dinvb.astype(BF16),
            "ident": ident.astype(BF16),
            "sall": Sm,
            "idxA": wrap_idxs(idx_streams[0]),
            "idxB": wrap_idxs(idx_streams[1]),
        })
    return in_maps


def postprocess(P, results):
    out = np.zeros((P.N, P.DOUT), np.float32)
    for c in range(P.NC):
        blk = results[c]["out"]  # [DOUT, BLK]
        out[c * P.NPC:(c + 1) * P.NPC] = blk[:, :P.NPC].T
    return out


# ---------------------------------------------------------------- builder
def build(P):
    import concourse.bacc as bacc
    import concourse.tile as tile
    import concourse.mybir as mybir

    dt = mybir.dt
    NC, BLK, SUB = P.NC, P.BLK, P.SUB
    F, HID, DOUT = P.F, P.HID, P.DOUT
    SA16 = P.SLOTS[0] // 16
    SB16 = P.SLOTS[1] // 16
    NQ = 4

    nc = bacc.Bacc("TRN2", target_bir_lowering=False, debug=False,
                   num_devices=NC, num_swdge_queues=NQ,
                   dynamic_dma_scratch_size=12288)
    xrows_d = nc.dram_tensor("xrows", [P.NA + P.NB, F], dt.bfloat16,
                             kind="ExternalInput")
    xTown_d = nc.dram_tensor("xTown", [F, BLK], dt.bfloat16,
                             kind="ExternalInput")
    w1_d = nc.dram_tensor("w1", [F, HID], dt.bfloat16, kind="ExternalInput")
    w2_d = nc.dram_tensor("w2", [HID, DOUT], dt.bfloat16,
                          kind="ExternalInput")
    b1_d = nc.dram_tensor("b1", [HID, 1], dt.float32, kind="ExternalInput")
    b2_d = nc.dram_tensor("b2", [DOUT, 1], dt.float32, kind="ExternalInput")
    dinvb_d = nc.dram_tensor("dinvb", [128, BLK], dt.bfloat16,
                             kind="ExternalInput")
    ident_d = nc.dram_tensor("ident", [128, 128], dt.bfloat16,
                             kind="ExternalInput")
    SCOLS = P.TOTCH * SUB
    MAXSW = max((P.NCH[wi][0] + P.NCH[wi][1]) * SUB
                for wi in range(len(P.windows)))
    sall_d = nc.dram_tensor("sall", [128, SCOLS], dt.bfloat16,
                            kind="ExternalInput")
    idxA_d = nc.dram_tensor("idxA", [128, SA16], dt.int16,
                            kind="ExternalInput")
    idxB_d = nc.dram_tensor("idxB", [128, SB16], dt.int16,
                            kind="ExternalInput")
    out_d = nc.dram_tensor("out", [DOUT, BLK], dt.float32,
                           kind="ExternalOutput")

    with tile.TileContext(nc) as tc:
        with (
            tc.tile_pool(name="dram", bufs=1, space="DRAM") as dram,
            tc.tile_pool(name="const", bufs=1) as cpool,
            tc.tile_pool(name="msgs", bufs=5) as mpool,
            tc.tile_pool(name="smat", bufs=2) as spool,
            tc.tile_pool(name="zxb", bufs=3) as zpool,
            tc.tile_pool(name="drain", bufs=3) as drpool,
            tc.tile_pool(name="rows", bufs=3) as rpool,
            tc.tile_pool(name="psum_z", bufs=3, space="PSUM") as pz,
            tc.tile_pool(name="psum_h", bufs=2, space="PSUM") as ph,
            tc.tile_pool(name="psum_tp", bufs=2, space="PSUM") as ptp,
            tc.tile_pool(name="psum_po", bufs=1, space="PSUM") as ppo,
        ):
            ag_in = dram.tile([BLK, HID], dt.bfloat16)
            table2a = dram.tile([P.NA, HID], dt.bfloat16,
                                addr_space="Shared")
            table2b = dram.tile([P.NB, HID], dt.bfloat16,
                                addr_space="Shared")

            # ---- constants to SBUF
            w1sb = cpool.tile([F, HID], dt.bfloat16)
            nc.sync.dma_start(w1sb[:], w1_d[:])
            w2sb = cpool.tile([HID, DOUT], dt.bfloat16)
            nc.sync.dma_start(w2sb[:], w2_d[:])
            b1sb = cpool.tile([HID, 1], dt.float32)
            nc.sync.dma_start(b1sb[:], b1_d[:])
            b2sb = cpool.tile([DOUT, 1], dt.float32)
            nc.sync.dma_start(b2sb[:], b2_d[:])
            dinvb = cpool.tile([128, BLK], dt.bfloat16)
            nc.sync.dma_start(dinvb[:], dinvb_d[:])
            ident = cpool.tile([128, 128], dt.bfloat16)
            nc.sync.dma_start(ident[:], ident_d[:])

            idxA = cpool.tile([128, SA16], dt.int16)
            nc.sync.dma_start(idxA[:], idxA_d[:])
            idxB = cpool.tile([128, SB16], dt.int16)
            nc.sync.dma_start(idxB[:], idxB_d[:])
            xts = cpool.tile([128, BLK], dt.bfloat16)
            nc.sync.dma_start(xts[:], xTown_d[:])
            h2b = cpool.tile([128, BLK], dt.bfloat16)

            def ag_range(lo, n, dst_tile):
                nc.gpsimd.collective_compute(
                    "AllGather",
                    mybir.AluOpType.bypass,
                    ins=[ag_in[lo:lo + n, :].opt()],
                    outs=[dst_tile[:].opt()],
                    replica_groups=[list(range(NC))],
                )

            # ---- flat job list over both layers: job = [layer, wi, s, nch, q]
            jobs = []
            qctr = 0
            for layer in (1, 2):
                for wi in range(len(P.windows)):
                    for s in (0, 1):
                        nch = P.NCH[wi][s]
                        if nch == 0:
                            continue
                        jobs.append([layer, wi, s, nch, qctr % NQ])
                        qctr += 1
            njobs = len(jobs)

            l1_ioffs = {}
            offs16 = [0, 0]
            for wi in range(len(P.windows)):
                for s in (0, 1):
                    nch = P.NCH[wi][s]
                    if nch == 0:
                        continue
                    l1_ioffs[(wi, s)] = offs16[s]
                    offs16[s] += nch * 8

            job_msgs = [None] * njobs

            def emit_gather(j):
                if job_msgs[j] is not None:
                    return
                layer, wi, s, nch, q = jobs[j]
                idx = idxA if s == 0 else idxB
                ioff = l1_ioffs[(wi, s)]
                if layer == 1:
                    tab = (xrows_d[0:P.NA, :] if s == 0
                           else xrows_d[P.NA:P.NA + P.NB, :])
                else:
                    tab = table2a[:] if s == 0 else table2b[:]
                mt = mpool.tile([128, nch, F], dt.bfloat16,
                                tag=f"m{s}", name=f"msgs_l{layer}_{wi}_{s}")
                job_msgs[j] = mt
                nc.gpsimd.dma_gather(
                    mt[:], tab,
                    idx[:, ioff: ioff + nch * 8],
                    nch * 128, nch * 128, F,
                    single_packet=False,
                    queue_num=q,
                )

            win_jobs = {}
            for j, (layer, wi, s, nch, q) in enumerate(jobs):
                win_jobs.setdefault((layer, wi), []).append(j)

            win_state = {}

            def emit_mms(layer, wi):
                w = P.windows[wi]
                cols = slice(w.col0, w.col0 + w.ncols)
                ncols = w.ncols
                nchA, nchB = P.NCH[wi]
                nch_tot = nchA + nchB
                soff = 0
                for wj in range(wi):
                    soff += P.NCH[wj][0] + P.NCH[wj][1]

                for j in win_jobs[(layer, wi)]:
                    emit_gather(j)

                zw = pz.tile([128, P.WIN], dt.float32, tag="z")
                src_self = xts if layer == 1 else h2b
                nc.tensor.matmul(
                    zw[:, :ncols], ident[:], src_self[:, cols],
                    start=True, stop=False,
                )

                swt = spool.tile([128, MAXSW], dt.bfloat16, tag="sw")
                nc.sync.dma_start(
                    swt[:, : nch_tot * SUB],
                    sall_d[:, soff * SUB:(soff + nch_tot) * SUB],
                )

                k_all = 0
                for s in (0, 1):
                    jlist = [j for j in win_jobs[(layer, wi)]
                             if jobs[j][2] == s]
                    if not jlist:
                        continue
                    mt = job_msgs[jlist[0]]
                    for k, base in enumerate(P.plan[wi][s]):
                        k_all += 1
                        nc.tensor.matmul(
                            zw[:, base:base + SUB],
                            mt[:, k, :],
                            swt[:, (k_all - 1) * SUB:k_all * SUB],
                            start=False, stop=(k_all == nch_tot),
                        )

                # free zw fast (off the PE stream) so the PSUM window pool
                # rotates; the rest of the tail is deferred one window.
                if layer == 1:
                    zxb = zpool.tile([128, P.WIN], dt.bfloat16, tag="zx")
                    nc.vector.tensor_copy(zxb[:, :ncols], zw[:, :ncols])
                    win_state[(layer, wi)] = zxb
                else:
                    qT = drpool.tile([128, P.WIN], dt.bfloat16, tag="qT")
                    nc.vector.tensor_tensor(
                        qT[:, :ncols], zw[:, :ncols], dinvb[:, cols],
                        op=mybir.AluOpType.mult,
                    )
                    po = ppo.tile([DOUT, P.WIN], dt.float32, tag="po")
                    nc.tensor.matmul(
                        po[:, :ncols], w2sb[:], qT[:, :ncols],
                        start=True, stop=True,
                    )
                    ot = rpool.tile([DOUT, P.WIN], dt.float32, tag="ot")
                    nc.scalar.activation(
                        ot[:, :ncols], po[:, :ncols],
                        mybir.ActivationFunctionType.Relu, bias=b2sb[:],
                    )
                    nc.sync.dma_start(out_d[:, cols], ot[:, :ncols])

            def emit_tail(layer, wi):
                w = P.windows[wi]
                cols = slice(w.col0, w.col0 + w.ncols)
                ncols = w.ncols
                if layer == 1:
                    zxb = win_state.pop((layer, wi))
                    z1 = ph.tile([128, P.WIN], dt.float32, tag="z1")
                    nc.tensor.matmul(
                        z1[:, :ncols], w1sb[:], zxb[:, :ncols],
                        start=True, stop=True,
                    )
                    t1 = drpool.tile([128, P.WIN], dt.float32, tag="t1")
                    nc.vector.tensor_tensor(
                        t1[:, :ncols], z1[:, :ncols], dinvb[:, cols],
                        op=mybir.AluOpType.mult,
                    )
                    t2 = drpool.tile([128, P.WIN], dt.float32, tag="t2")
                    nc.scalar.activation(
                        t2[:, :ncols], t1[:, :ncols],
                        mybir.ActivationFunctionType.Relu, bias=b1sb[:],
                    )
                    nc.vector.tensor_tensor(
                        h2b[:, cols], t2[:, :ncols], dinvb[:, cols],
                        op=mybir.AluOpType.mult,
                    )
                    for jc in range(0, ncols, 128):
                        nj = min(128, ncols - jc)
                        tp = ptp.tile([128, 128], dt.bfloat16, tag="tp")
                        nc.tensor.transpose(
                            tp[:nj, :],
                            h2b[:, w.col0 + jc: w.col0 + jc + nj],
                            ident[:]
                        )
                        hr = rpool.tile([128, 128], dt.bfloat16,
                                        tag="hr")
                        if (jc // 128) % 2 == 0:
                            nc.vector.tensor_copy(hr[:nj, :], tp[:nj, :])
                        else:
                            nc.scalar.copy(hr[:nj, :], tp[:nj, :])
                        nc.sync.dma_start(
                            ag_in[w.col0 + jc: w.col0 + jc + nj, :],
                            hr[:nj, :]
                        )

            nwin = len(P.windows)
            pending = None
            for wi in range(nwin):
                emit_mms(1, wi)
                if pending is not None:
                    emit_tail(1, pending)
                    if pending == P.AG_SPLIT_WIN - 1:
                        ag_range(0, P.ASPL, table2a)
                pending = wi
            emit_tail(1, pending)
            ag_range(P.ASPL, P.BSPL, table2b)
            for wi in range(nwin):
                emit_mms(2, wi)

    nc.compile()
    return nc


# ----------------------------------------------------------------- kernel()
_BUILD_CACHE = {}
_LAST = {}


def _get_nc(P, key, **bkw):
    ent = _BUILD_CACHE.get(key)
    if ent is None:
        ent = build(P, **bkw)
        _BUILD_CACHE[key] = ent
    return ent


def kernel(x, edge_index, W1, b1, W2, b2):
    import numpy as np
    x = np.asarray(x)
    edge_index = np.asarray(edge_index)
    N = x.shape[0]
    NC = 8
    P = make_structure(N, NC)
    in_maps = prep(P, x, edge_index, np.asarray(W1), np.asarray(b1),
                   np.asarray(W2), np.asarray(b2))
    key = (N, x.shape[1], np.asarray(W2).shape[1], P.TOTCH,
           tuple(tuple(n) for n in P.NCH))
    nc = _get_nc(P, key)
    _LAST.update(P=P, in_maps=in_maps, nc=nc)
    from concourse.bass_utils import run_bass_kernel_spmd
    res = run_bass_kernel_spmd(nc, in_maps, core_ids=list(range(NC)))
    return postprocess(P, res.results).astype(np.float32)
